# revision 24
# baseline (speedup 1.0000x reference)
"""HNetv1 Trainium2 Bass kernel (v3 — pipelined, queue-separated).

Strategy (8 NeuronCores):
  - Warmup AllGather issued at t~0 absorbs SPMD launch skew / first-collective
    rendezvous while compute proceeds.
  - PE warmup matmuls run under the x1/x2 input DMAs so the HAM clock gate
    reaches 2.4 GHz before the corr matmuls start.
  - Every core computes the l2-normalized correlation X = [64, 20736] for all
    batches (needed in full for column-parallel L1); elementwise norm work is
    batched in 16-batch superblocks to amortize per-op engine overhead.
  - w1 column-slice per core streamed from HBM in [128, 9, 216] bf16 tiles
    (per-partition-contiguous layout -> large DMA descriptors), prefetched
    during the corr phase via a deep tile pool. w1 DMAs own the Sync (SP)
    HWDGE queue; weight/bias loads go on the Scalar (ACT) HWDGE queue; loads
    that wait on collectives go on the GpSimd (SWDGE) queue so nothing blocks
    the w1 stream.
  - L1 is split into 3 column chunks of 216: each chunk's transpose+AllGather
    overlaps the next chunk's matmuls; L2 accumulates all gathered chunks
    (a zero-contribution matmul pins L2 behind the last chunk's transposes
    in the PE stream so the final AllGather launches as early as possible).
  - L2 column-split + AllGather, L3/L4 redundant on every core.

Layouts (validated against the reference in numpy):
  x1r/x2r: [C=128, N=64, HW=144] bf16 (host-transposed).
  corr^T for batch n is produced as psum [ij, k]; X_a[ij 0:128, k, n] holds the
  main part, the ij residue (16 rows) is staged in X_btmp[16, k, n] and
  regrouped by an SBUF->SBUF DMA into X_b[16*bi+r, bo, n] with k = 8*bo+bi.
  w1 per core/chunk: [128, 162, 324] where tile t<144 is w1[(k=t)*144+ij] rows
  ij 0..127 and tiles 144+bo hold the gathered residue rows — L1 is 162 plain
  [128,64]x[128,324] accumulating matmuls per chunk.
"""

import os
import numpy as np
import ml_dtypes

N, C, S = 64, 128, 12
HW = S * S            # 144
RIN = S ** 4          # 20736
NCORES = 8
COLS1 = 5184 // NCORES   # 648
COLS2 = 1296 // NCORES   # 162
G = 4                    # batches per corr psum group
NGRP = N // G            # 16
SB = 16                  # batches per corr superblock (elementwise granularity)
NSB = N // SB            # 4
NCH = 3                  # L1 column chunks
CCOLS = COLS1 // NCH     # 324
CPAD = 256               # padded h1T rows per rank per chunk (2*128)
PAD2 = 256               # per-rank padded h2T rows (2*128)
TB = 9                   # w1 k-tiles per DMA (5832B/partition descriptors)
NTB = 162 // TB          # 9 DMAs per chunk

_CACHE = {}

LAST_RESULT = None  # BassKernelResults from the most recent run (for test.py)


def _bf16(a):
    return np.asarray(a, dtype=np.float32).astype(ml_dtypes.bfloat16)


def _build_nc(trace_enabled=False):
    import concourse.bacc as bacc
    import concourse.tile as tile
    import concourse.mybir as mybir
    from concourse.masks import make_identity

    dt = mybir.dt
    AF = mybir.ActivationFunctionType
    ALU = mybir.AluOpType

    nc = bacc.Bacc("TRN2", target_bir_lowering=False, debug=False,
                   num_devices=NCORES)

    x1r_d = nc.dram_tensor("x1r", [C, N, HW], dt.bfloat16, kind="ExternalInput")
    x2r_d = nc.dram_tensor("x2r", [C, N, 18], dt.bfloat16, kind="ExternalInput")
    w1c_d = [nc.dram_tensor(f"w1c{c}", [128, 162, CCOLS], dt.bfloat16,
                            kind="ExternalInput") for c in range(NCH)]
    b1s_d = nc.dram_tensor("b1s", [1, COLS1], dt.bfloat16, kind="ExternalInput")
    w2c_d = [nc.dram_tensor(f"w2c{c}", [128, 2 * NCORES, COLS2], dt.bfloat16,
                            kind="ExternalInput") for c in range(NCH)]
    b2s_d = nc.dram_tensor("b2s", [1, COLS2], dt.bfloat16, kind="ExternalInput")
    w3p_d = nc.dram_tensor("w3p", [128, 16, 324], dt.bfloat16, kind="ExternalInput")
    b3_d = nc.dram_tensor("b3r", [1, 324], dt.bfloat16, kind="ExternalInput")
    w4p_d = nc.dram_tensor("w4p", [128, 3, 8], dt.bfloat16, kind="ExternalInput")
    b4_d = nc.dram_tensor("b4r", [1, 8], dt.bfloat16, kind="ExternalInput")
    out_d = nc.dram_tensor("out", [N, 8], dt.float32, kind="ExternalOutput")

    rg = [list(range(NCORES))]

    with tile.TileContext(nc) as tc:
        with tc.tile_pool(name="persist", bufs=1) as persist, \
             tc.tile_pool(name="dramp", bufs=1, space="DRAM") as dramp:
            # internal DRAM for collectives
            wu_in = dramp.tile([1, 16], dt.bfloat16)
            wu_out = dramp.tile([NCORES, 16], dt.bfloat16, addr_space="Shared")
            ag1_in = [dramp.tile([128, 2, N], dt.bfloat16, name=f"ag1_in{c}")
                      for c in range(NCH)]
            ag1_out = [dramp.tile([NCORES * 128, 2, N], dt.bfloat16,
                                  addr_space="Shared", name=f"ag1_out{c}")
                       for c in range(NCH)]
            agx_in = dramp.tile([128, 18, N], dt.bfloat16)
            agx_out = dramp.tile([NCORES * 128, 18, N], dt.bfloat16, addr_space="Shared")
            agr_in = dramp.tile([16, 18, N], dt.bfloat16)
            agr_out = dramp.tile([NCORES * 16, 18, N], dt.bfloat16, addr_space="Shared")
            ag2_in = dramp.tile([128, 2, N], dt.bfloat16)
            ag2_out = dramp.tile([NCORES * 128, 2, N], dt.bfloat16, addr_space="Shared")

            # warmup collective: absorbs SPMD launch skew + ncfw rendezvous
            # while the corr phase computes.
            wtiny = persist.tile([1, 16], dt.bfloat16)
            with tc.high_priority():
                nc.vector.memset(wtiny[:], 0.0)
                nc.gpsimd.dma_start(wu_in[:], wtiny[:])
                nc.gpsimd.collective_compute(
                    "AllGather", mybir.AluOpType.bypass, replica_groups=rg,
                    ins=[wu_in[:]], outs=[wu_out[:]])

            ones128 = persist.tile([128, 128], dt.bfloat16)
            nc.vector.memset(ones128[:], 1.0)
            onesrow = persist.tile([1, N], dt.bfloat16)
            nc.vector.memset(onesrow[:], 1.0)
            ident = persist.tile([128, 128], dt.bfloat16)
            make_identity(nc, ident[:])
            warm = persist.tile([128, 512], dt.bfloat16)
            nc.vector.memset(warm[:], 0.0)
            nc.scalar.activation(warm[0:1, 0:16], warm[0:1, 0:16], AF.Tanh)

            # x1 full + the k%8==rank slice of x2 resident in SBUF, loaded
            # per-superblock so corr starts as soon as the first slice lands
            x1sb = persist.tile([C, N, HW], dt.bfloat16)
            x2sb = persist.tile([C, N, 18], dt.bfloat16)
            nc.sync.dma_start(x2sb[:], x2r_d[:, :, :])
            for sb in range(NSB):
                n0 = SB * sb
                nc.sync.dma_start(x1sb[:, n0:n0 + SB, :], x1r_d[:, n0:n0 + SB, :])

            # PE warmup: ~6us of dummy matmuls under the x1/x2 DMAs flips the
            # HAM clock gate to 2.4 GHz before the corr matmuls arrive.
            with tc.tile_pool(name="pwarm", bufs=1, space="PSUM") as pwarm:
                warmps = pwarm.tile([128, 512], dt.float32, tag="warmps")
                for _ in range(14):
                    nc.tensor.matmul(warmps[:], ones128[:], warm[:],
                                     start=True, stop=True)

            Xag = persist.tile([128, 18, N], dt.bfloat16)     # [ij, bo, n] (local k=8*bo+rank)
            Xres = persist.tile([16, 18, N], dt.bfloat16)     # [ij-128, bo, n]
            X_a = persist.tile([128, NCORES, 18, N], dt.bfloat16)  # [ij, k%8, k//8, n]
            X_b = persist.tile([128, 18, N], dt.bfloat16)     # [16*(k%8)+r, k//8, n]

            # h1T pad rows must be zero; memset early (cheap, off critical path)
            h1T = [persist.tile([128, 2, N], dt.bfloat16, name=f"h1T{c}")
                   for c in range(NCH)]
            for c in range(NCH):
                nc.vector.memset(h1T[c][:], 0.0)
            h2T = persist.tile([128, 2, N], dt.bfloat16)
            nc.vector.memset(h2T[:], 0.0)
            h3T = persist.tile([128, 3, N], dt.bfloat16)
            nc.vector.memset(h3T[:], 0.0)

            # w1 stream pool opened for the whole kernel so prefetch starts
            # during the corr phase (14 bufs x 0.75 MB lookahead).
            with tc.tile_pool(name="w1p", bufs=14) as w1p:
                # ------- corr phase: k-split (this rank owns k%8==rank) -------
                with tc.tile_pool(name="csq", bufs=2) as csq, \
                     tc.tile_pool(name="pssq", bufs=1, space="PSUM") as pssq, \
                     tc.tile_pool(name="pss2", bufs=1, space="PSUM") as pss2, \
                     tc.tile_pool(name="pca", bufs=2, space="PSUM") as pca, \
                     tc.tile_pool(name="pcb", bufs=1, space="PSUM") as pcb:
                    for blk in range(NSB):
                        n0 = SB * blk
                        x1t = x1sb[:, n0:n0 + SB, :]
                        x2t = x2sb[:, n0:n0 + SB, :]

                        sq1 = csq.tile([C, SB, HW], dt.bfloat16, tag="sq1")
                        nc.vector.tensor_tensor(sq1[:], x1t, x1t, ALU.mult)
                        sq2 = csq.tile([C, SB, 18], dt.bfloat16, tag="sq2")
                        nc.vector.tensor_tensor(sq2[:], x2t, x2t, ALU.mult)

                        # rsqrt of the partition-broadcast ssq, bf16 out
                        r1 = csq.tile([128, SB, HW], dt.bfloat16, tag="r1")
                        for q in range(2):
                            b0 = 8 * q
                            ssq = pssq.tile([128, 4, 512], dt.float32, tag="ssq")
                            for h in range(4):
                                nc.tensor.matmul(ssq[:, h, 0:2 * HW], ones128[:],
                                                 sq1[:, b0 + 2 * h:b0 + 2 * h + 2, :],
                                                 start=True, stop=True)
                            nc.scalar.activation(
                                r1[:, b0:b0 + 4, :].rearrange("p (a b) k -> p a (b k)", b=2),
                                ssq[:, 0:2, 0:2 * HW], AF.Abs_reciprocal_sqrt)
                            nc.scalar.activation(
                                r1[:, b0 + 4:b0 + 8, :].rearrange("p (a b) k -> p a (b k)", b=2),
                                ssq[:, 2:4, 0:2 * HW], AF.Abs_reciprocal_sqrt)
                        ssq2 = pss2.tile([128, 512], dt.float32, tag="ssq2")
                        nc.tensor.matmul(ssq2[:, 0:SB * 18], ones128[:],
                                         sq2[:].rearrange("p n k -> p (n k)"),
                                         start=True, stop=True)
                        r2 = csq.tile([128, SB, 18], dt.bfloat16, tag="r2")
                        nc.scalar.activation(
                            r2[:].rearrange("p n k -> p (n k)"),
                            ssq2[:, 0:SB * 18], AF.Abs_reciprocal_sqrt)

                        x1s = csq.tile([C, SB, HW], dt.bfloat16, tag="x1s")
                        nc.vector.tensor_tensor(x1s[:], x1t, r1[:], ALU.mult)
                        x2s = csq.tile([C, SB, 18], dt.bfloat16, tag="x2s")
                        nc.vector.tensor_tensor(x2s[:], x2t, r2[:], ALU.mult)

                        ca = pca.tile([128, SB, 18], dt.float32, tag="ca")
                        cb = pcb.tile([16, SB, 18], dt.float32, tag="cb")
                        for b in range(SB):
                            nc.tensor.matmul(ca[:, b, :], x1s[:, b, 0:128],
                                             x2s[:, b, :], start=True, stop=True)
                            nc.tensor.matmul(cb[:, b, :], x1s[:, b, 128:HW],
                                             x2s[:, b, :], start=True, stop=True)
                        nc.vector.tensor_copy(
                            Xag[:, :, n0:n0 + SB].rearrange("p k n -> p n k"), ca[:])
                        nc.scalar.copy(
                            Xres[:, :, n0:n0 + SB].rearrange("p k n -> p n k"), cb[:])

                    # AllGather X: main part and the ij-residue. The residue
                    # gather lands exactly in the X_b layout (k%8 == source
                    # rank), so no regrouping is needed.
                    nc.scalar.dma_start(agx_in[:], Xag[:])
                    nc.scalar.dma_start(agr_in[:], Xres[:])
                    nc.gpsimd.collective_compute(
                        "AllGather", mybir.AluOpType.bypass, replica_groups=rg,
                        ins=[agx_in[:]], outs=[agx_out[:]])
                    nc.gpsimd.collective_compute(
                        "AllGather", mybir.AluOpType.bypass, replica_groups=rg,
                        ins=[agr_in[:]], outs=[agr_out[:]])
                    nc.gpsimd.dma_start(
                        X_a[:], agx_out[:].rearrange("(r p) k n -> p r k n", p=128))
                    nc.gpsimd.dma_start(X_b[:], agr_out[:])

                # ---------------- L1 (2 column chunks) + AG + L2 ----------------
                h1sb = persist.tile([64, COLS1], dt.bfloat16)
                h2sb = persist.tile([64, COLS2], dt.bfloat16)
                xt2 = [persist.tile([128, NCORES, 2, N], dt.bfloat16, name=f"xt2_{c}")
                       for c in range(NCH)]
                w2sb = [persist.tile([128, 2 * NCORES, COLS2], dt.bfloat16, name=f"w2sb{c}")
                        for c in range(NCH)]

                with tc.tile_pool(name="bias", bufs=1) as biasp, \
                     tc.tile_pool(name="ph1", bufs=1, space="PSUM") as ph1, \
                     tc.tile_pool(name="ptp", bufs=2, space="PSUM") as ptp, \
                     tc.tile_pool(name="ph2", bufs=1, space="PSUM") as ph2:
                    b1row = biasp.tile([1, COLS1], dt.bfloat16, tag="b1")
                    nc.scalar.dma_start(b1row[:], b1s_d[:, :])
                    b2row = biasp.tile([1, COLS2], dt.bfloat16, tag="b2")
                    nc.scalar.dma_start(b2row[:], b2s_d[:, :])
                    for c in range(NCH):
                        nc.scalar.dma_start(w2sb[c][:], w2c_d[c][:, :, :])
                    h1ps = [ph1.tile([64, CCOLS], dt.float32, tag=f"h1ps{c}",
                                     name=f"h1ps{c}") for c in range(NCH)]
                    h2ps = ph2.tile([64, COLS2], dt.float32, tag="h2ps")

                    for c in range(NCH):
                        for tb in range(NTB):
                            w1t = w1p.tile([128, TB, CCOLS], dt.bfloat16, tag="w1t")
                            nc.sync.dma_start(
                                w1t[:], w1c_d[c][:, TB * tb:TB * tb + TB, :])
                            for j in range(TB):
                                t = TB * tb + j
                                lhsT = (X_a[:, t % 8, t // 8, :] if t < 144
                                        else X_b[:, t - 144, :])
                                nc.tensor.matmul(h1ps[c][:], lhsT, w1t[:, j, :],
                                                 start=(t == 0), stop=False)
                        with tc.high_priority():
                            nc.tensor.matmul(h1ps[c][:], onesrow[:],
                                             b1row[:, CCOLS * c:CCOLS * (c + 1)],
                                             start=False, stop=True)
                            nc.scalar.activation(h1sb[:, CCOLS * c:CCOLS * (c + 1)],
                                                 h1ps[c][:], AF.Relu)
                            # transpose chunk -> h1T[c][p, s, n] (col = 128*s+p)
                            for t in range(2):
                                w = 128 if t < 1 else CCOLS - 128  # 88
                                tp = ptp.tile([128, 64], dt.bfloat16, tag="tp")
                                nc.tensor.transpose(
                                    tp[0:w, :],
                                    h1sb[:, CCOLS * c + 128 * t:CCOLS * c + 128 * t + w],
                                    ident[0:64, 0:64])
                                nc.vector.tensor_copy(h1T[c][0:w, t, :], tp[0:w, :])
                            nc.sync.dma_start(ag1_in[c][:], h1T[c][:])
                            nc.gpsimd.collective_compute(
                                "AllGather", mybir.AluOpType.bypass, replica_groups=rg,
                                ins=[ag1_in[c][:]], outs=[ag1_out[c][:]])

                    for c in range(NCH):
                        nc.scalar.dma_start(
                            xt2[c][:],
                            ag1_out[c][:].rearrange("(r p) s n -> p r s n", p=128))

                    # L2: accumulate both gathered chunks. tile_wait_until
                    # pushes these after the chunk tails in the modeled
                    # schedule (the scheduler underestimates AG latency and
                    # would otherwise hoist L2 before chunk 1's transposes,
                    # stalling the PE queue on the collective).
                    # zero-contribution matmul (pad rows of w2 are zero) that
                    # reads h1T[1]: forces the PE stream to finish chunk 1's
                    # transposes before starting L2, so AG-c1 launches early.
                    nc.tensor.matmul(h2ps[:], h1T[NCH - 1][:, 1, :],
                                     warm[:, 0:COLS2], start=True, stop=False)
                    for c in range(NCH):
                        for tt in range(2 * NCORES):
                            nc.tensor.matmul(h2ps[:], xt2[c][:, tt // 2, tt % 2, :],
                                             w2sb[c][:, tt, :],
                                             start=False, stop=False)
                    nc.tensor.matmul(h2ps[:], onesrow[:], b2row[:],
                                     start=False, stop=True)
                    nc.scalar.activation(h2sb[:], h2ps[:], AF.Relu)

                    # transpose h2 -> [162(+pad), 64] and AllGather
                    for t in range(2):
                        w = 128 if t < 1 else COLS2 - 128  # 34
                        tp = ptp.tile([128, 64], dt.bfloat16, tag="tp")
                        nc.tensor.transpose(tp[0:w, :], h2sb[:, 128 * t:128 * t + w],
                                            ident[0:64, 0:64])
                        nc.vector.tensor_copy(h2T[0:w, t, :], tp[0:w, :])
                    nc.sync.dma_start(ag2_in[:], h2T[:])
                    nc.gpsimd.collective_compute(
                        "AllGather", mybir.AluOpType.bypass, replica_groups=rg,
                        ins=[ag2_in[:]], outs=[ag2_out[:]])

                # ---------------- L3 (redundant) ----------------
                h3sb = persist.tile([64, 324], dt.bfloat16)
                with tc.tile_pool(name="l3", bufs=1) as l3p, \
                     tc.tile_pool(name="ph3", bufs=1, space="PSUM") as ph3:
                    w3sb = l3p.tile([128, 16, 324], dt.bfloat16, tag="w3sb")
                    nc.scalar.dma_start(w3sb[:], w3p_d[:, :, :])
                    b3row = l3p.tile([1, 324], dt.bfloat16, tag="b3")
                    nc.scalar.dma_start(b3row[:], b3_d[:, :])
                    xt3 = l3p.tile([128, NCORES, 2, N], dt.bfloat16, tag="xt3")
                    nc.gpsimd.dma_start(
                        xt3[:], ag2_out[:].rearrange("(r p) s n -> p r s n", p=128))
                    h3ps = ph3.tile([64, 324], dt.float32, tag="h3ps")
                    for t in range(16):
                        nc.tensor.matmul(h3ps[:], xt3[:, t // 2, t % 2, :], w3sb[:, t, :],
                                         start=(t == 0), stop=False)
                    nc.tensor.matmul(h3ps[:], onesrow[:], b3row[:], start=False, stop=True)
                    nc.scalar.activation(h3sb[:], h3ps[:], AF.Tanh)

                # ---------------- L4 (redundant) ----------------
                with tc.tile_pool(name="ptp3", bufs=2, space="PSUM") as ptp3, \
                     tc.tile_pool(name="l4", bufs=1) as l4p, \
                     tc.tile_pool(name="ph4", bufs=1, space="PSUM") as ph4:
                    for t in range(3):
                        w = 128 if t < 2 else 324 - 256  # 68
                        tp = ptp3.tile([128, 64], dt.bfloat16, tag="tp3")
                        nc.tensor.transpose(tp[0:w, :], h3sb[:, 128 * t:128 * t + w],
                                            ident[0:64, 0:64])
                        nc.vector.tensor_copy(h3T[0:w, t, :], tp[0:w, :])
                    w4sb = l4p.tile([128, 3, 8], dt.bfloat16, tag="w4sb")
                    nc.scalar.dma_start(w4sb[:], w4p_d[:, :, :])
                    b4row = l4p.tile([1, 8], dt.bfloat16, tag="b4")
                    nc.scalar.dma_start(b4row[:], b4_d[:, :])
                    outps = ph4.tile([64, 8], dt.float32, tag="outps")
                    for t in range(3):
                        nc.tensor.matmul(outps[:], h3T[:, t, :], w4sb[:, t, :],
                                         start=(t == 0), stop=False)
                    nc.tensor.matmul(outps[:], onesrow[:], b4row[:], start=False, stop=True)
                    outsb = l4p.tile([64, 8], dt.float32, tag="outsb")
                    nc.vector.tensor_copy(outsb[:], outps[:])
                    nc.scalar.dma_start(out_d[:, :], outsb[:])

    nc.compile()
    return nc


def _prep_inputs(x1, x2, w1, b1, w2, b2, w3, b3, w4, b4):
    """Host-side shard/permute/cast. Returns per-core input maps."""
    x1f = np.asarray(x1, np.float32).reshape(N, C, HW)
    x2f = np.asarray(x2, np.float32).reshape(N, C, HW)
    x1r = _bf16(np.ascontiguousarray(x1f.transpose(1, 0, 2)))
    x2r = _bf16(np.ascontiguousarray(x2f.transpose(1, 0, 2)))
    w1 = np.asarray(w1, np.float32)
    w2 = np.asarray(w2, np.float32)
    w3 = np.asarray(w3, np.float32)
    w4 = np.asarray(w4, np.float32)

    # w2 column-slice per core; rows padded to the chunked-AG layout:
    # chunk c, rank r, sub-tile s in 0..2, partition p ->
    #   w2 row 648*r + 324*c + 128*s + p  (zero when 128*s+p >= 324)
    w2pad = np.zeros((NCH, NCORES, 2, 128, 1296), np.float32)
    for cc in range(NCH):
        for r in range(NCORES):
            blk = w2[COLS1 * r + CCOLS * cc: COLS1 * r + CCOLS * (cc + 1)]  # [216, 1296]
            w2pad[cc, r].reshape(CPAD, 1296)[:CCOLS] = blk
    # -> per chunk: [128, 16, 1296] with tile index tt = 2*r + s
    w2t = [np.ascontiguousarray(
        w2pad[cc].reshape(2 * NCORES, 128, 1296).transpose(1, 0, 2))
        for cc in range(NCH)]

    # w3 padded to PAD2-row rank blocks, transposed to [128, 16, 324]
    w3pad = np.zeros((NCORES, PAD2, 324), np.float32)
    for r in range(NCORES):
        w3pad[r, :COLS2] = w3[COLS2 * r:COLS2 * (r + 1)]
    w3t = np.ascontiguousarray(
        w3pad.reshape(16, 128, 324).transpose(1, 0, 2))

    w4pad = np.zeros((384, 8), np.float32)
    w4pad[:324] = w4
    w4t = np.ascontiguousarray(w4pad.reshape(3, 128, 8).transpose(1, 0, 2))

    in_maps = []
    for core in range(NCORES):
        w1c = w1[:, COLS1 * core:COLS1 * (core + 1)].reshape(HW, HW, COLS1)
        main = w1c[:, 0:128, :]
        res = w1c[:, 128:HW, :].reshape(18, 8, 16, COLS1).reshape(18, 128, COLS1)
        w1full = np.concatenate([main, res], axis=0).transpose(1, 0, 2)  # [128,162,648]
        m = {
            "x1r": x1r,
            "x2r": np.ascontiguousarray(x2r[:, :, core::8]),
            "b1s": _bf16(b1[COLS1 * core:COLS1 * (core + 1)]).reshape(1, COLS1),
            "b2s": _bf16(b2[COLS2 * core:COLS2 * (core + 1)]).reshape(1, COLS2),
            "w3p": _bf16(w3t),
            "b3r": _bf16(b3).reshape(1, 324),
            "w4p": _bf16(w4t),
            "b4r": _bf16(b4).reshape(1, 8),
        }
        for cc in range(NCH):
            m[f"w1c{cc}"] = _bf16(np.ascontiguousarray(
                w1full[:, :, CCOLS * cc:CCOLS * (cc + 1)]))
            m[f"w2c{cc}"] = _bf16(np.ascontiguousarray(
                w2t[cc][:, :, COLS2 * core:COLS2 * (core + 1)]))
        in_maps.append(m)
    return in_maps


def kernel(x1, x2, w1, b1, w2, b2, w3, b3, w4, b4):
    global LAST_RESULT
    from concourse.bass_utils import run_bass_kernel_spmd

    if "nc" not in _CACHE:
        _CACHE["nc"] = _build_nc()
    nc = _CACHE["nc"]

    in_maps = _prep_inputs(x1, x2, w1, b1, w2, b2, w3, b3, w4, b4)
    trace = bool(int(os.environ.get("HNET_TRACE", "0")))
    res = run_bass_kernel_spmd(nc, in_maps, core_ids=list(range(NCORES)),
                               trace=trace)
    LAST_RESULT = res
    H = np.asarray(res.results[0]["out"], np.float32)
    ones = np.ones((N, 1), np.float32)
    return np.concatenate([H, ones], axis=1).reshape(N, 3, 3)


# revision 25
# speedup vs baseline: 1.0732x; 1.0732x over previous
"""HNetv1 Trainium2 Bass kernel (v3 — pipelined, queue-separated).

Strategy (8 NeuronCores):
  - Warmup AllGather issued at t~0 absorbs SPMD launch skew / first-collective
    rendezvous while compute proceeds.
  - PE warmup matmuls run under the x1/x2 input DMAs so the HAM clock gate
    reaches 2.4 GHz before the corr matmuls start.
  - Every core computes the l2-normalized correlation X = [64, 20736] for all
    batches (needed in full for column-parallel L1); elementwise norm work is
    batched in 16-batch superblocks to amortize per-op engine overhead.
  - w1 column-slice per core streamed from HBM in [128, 9, 216] bf16 tiles
    (per-partition-contiguous layout -> large DMA descriptors), prefetched
    during the corr phase via a deep tile pool. w1 DMAs own the Sync (SP)
    HWDGE queue; weight/bias loads go on the Scalar (ACT) HWDGE queue; loads
    that wait on collectives go on the GpSimd (SWDGE) queue so nothing blocks
    the w1 stream.
  - L1 is split into 3 column chunks of 216: each chunk's transpose+AllGather
    overlaps the next chunk's matmuls; L2 accumulates all gathered chunks
    (a zero-contribution matmul pins L2 behind the last chunk's transposes
    in the PE stream so the final AllGather launches as early as possible).
  - L2 column-split + AllGather, L3/L4 redundant on every core.

Layouts (validated against the reference in numpy):
  x1r/x2r: [C=128, N=64, HW=144] bf16 (host-transposed).
  corr^T for batch n is produced as psum [ij, k]; X_a[ij 0:128, k, n] holds the
  main part, the ij residue (16 rows) is staged in X_btmp[16, k, n] and
  regrouped by an SBUF->SBUF DMA into X_b[16*bi+r, bo, n] with k = 8*bo+bi.
  w1 per core/chunk: [128, 162, 324] where tile t<144 is w1[(k=t)*144+ij] rows
  ij 0..127 and tiles 144+bo hold the gathered residue rows — L1 is 162 plain
  [128,64]x[128,324] accumulating matmuls per chunk.
"""

import os
import numpy as np
import ml_dtypes

N, C, S = 64, 128, 12
HW = S * S            # 144
RIN = S ** 4          # 20736
NCORES = 8
COLS1 = 5184 // NCORES   # 648
COLS2 = 1296 // NCORES   # 162
G = 4                    # batches per corr psum group
NGRP = N // G            # 16
SB = 16                  # batches per corr superblock (elementwise granularity)
NSB = N // SB            # 4
NCH = 3                  # L1 column chunks
CCOLS = COLS1 // NCH     # 324
CPAD = 256               # padded h1T rows per rank per chunk (2*128)
PAD2 = 256               # per-rank padded h2T rows (2*128)
TB = 9                   # w1 k-tiles per DMA (5832B/partition descriptors)
NTB = 162 // TB          # 9 DMAs per chunk

_CACHE = {}

LAST_RESULT = None  # BassKernelResults from the most recent run (for test.py)


def _bf16(a):
    return np.asarray(a, dtype=np.float32).astype(ml_dtypes.bfloat16)


def _build_nc(trace_enabled=False):
    import concourse.bacc as bacc
    import concourse.tile as tile
    import concourse.mybir as mybir
    from concourse.masks import make_identity

    dt = mybir.dt
    AF = mybir.ActivationFunctionType
    ALU = mybir.AluOpType

    nc = bacc.Bacc("TRN2", target_bir_lowering=False, debug=False,
                   num_devices=NCORES)

    x1r_d = nc.dram_tensor("x1r", [C, N, HW], dt.bfloat16, kind="ExternalInput")
    x2r_d = nc.dram_tensor("x2r", [C, N, 18], dt.bfloat16, kind="ExternalInput")
    w1c_d = [nc.dram_tensor(f"w1c{c}", [128, 162, CCOLS], dt.bfloat16,
                            kind="ExternalInput") for c in range(NCH)]
    b1s_d = nc.dram_tensor("b1s", [1, COLS1], dt.bfloat16, kind="ExternalInput")
    w2c_d = [nc.dram_tensor(f"w2c{c}", [128, 2 * NCORES, COLS2], dt.bfloat16,
                            kind="ExternalInput") for c in range(NCH)]
    b2s_d = nc.dram_tensor("b2s", [1, COLS2], dt.bfloat16, kind="ExternalInput")
    w3p_d = nc.dram_tensor("w3p", [128, 16, 324], dt.bfloat16, kind="ExternalInput")
    b3_d = nc.dram_tensor("b3r", [1, 324], dt.bfloat16, kind="ExternalInput")
    w4p_d = nc.dram_tensor("w4p", [128, 3, 8], dt.bfloat16, kind="ExternalInput")
    b4_d = nc.dram_tensor("b4r", [1, 8], dt.bfloat16, kind="ExternalInput")
    out_d = nc.dram_tensor("out", [N, 8], dt.float32, kind="ExternalOutput")

    rg = [list(range(NCORES))]

    with tile.TileContext(nc) as tc:
        with tc.tile_pool(name="persist", bufs=1) as persist, \
             tc.tile_pool(name="dramp", bufs=1, space="DRAM") as dramp:
            # internal DRAM for collectives
            wu_in = dramp.tile([1, 16], dt.bfloat16)
            wu_out = dramp.tile([NCORES, 16], dt.bfloat16, addr_space="Shared")
            ag1_in = [dramp.tile([128, 2, N], dt.bfloat16, name=f"ag1_in{c}")
                      for c in range(NCH)]
            ag1_out = [dramp.tile([NCORES * 128, 2, N], dt.bfloat16,
                                  addr_space="Shared", name=f"ag1_out{c}")
                       for c in range(NCH)]
            agx_in = dramp.tile([128, 18, N], dt.bfloat16)
            agx_out = dramp.tile([NCORES * 128, 18, N], dt.bfloat16, addr_space="Shared")
            agr_in = dramp.tile([16, 18, N], dt.bfloat16)
            agr_out = dramp.tile([NCORES * 16, 18, N], dt.bfloat16, addr_space="Shared")
            ag2_in = dramp.tile([128, 2, N], dt.bfloat16)
            ag2_out = dramp.tile([NCORES * 128, 2, N], dt.bfloat16, addr_space="Shared")

            # warmup collective: absorbs SPMD launch skew + ncfw rendezvous
            # while the corr phase computes.
            # no writer for wu_in: the gathered bytes are never consumed,
            # so the collective can launch with zero input dependencies.
            with tc.high_priority():
                nc.gpsimd.collective_compute(
                    "AllGather", mybir.AluOpType.bypass, replica_groups=rg,
                    ins=[wu_in[:]], outs=[wu_out[:]])

            ones128 = persist.tile([128, 128], dt.bfloat16)
            nc.vector.memset(ones128[:], 1.0)
            onesrow = persist.tile([1, N], dt.bfloat16)
            nc.vector.memset(onesrow[:], 1.0)
            ident = persist.tile([128, 128], dt.bfloat16)
            make_identity(nc, ident[:])
            warm = persist.tile([128, 512], dt.bfloat16)
            nc.vector.memset(warm[:], 0.0)
            nc.scalar.activation(warm[0:1, 0:16], warm[0:1, 0:16], AF.Tanh)

            # x1 full + the k%8==rank slice of x2 resident in SBUF, loaded
            # per-superblock so corr starts as soon as the first slice lands
            x1sb = persist.tile([C, N, HW], dt.bfloat16)
            x2sb = persist.tile([C, N, 18], dt.bfloat16)
            nc.sync.dma_start(x2sb[:], x2r_d[:, :, :])
            for sb in range(NSB):
                n0 = SB * sb
                nc.sync.dma_start(x1sb[:, n0:n0 + SB, :], x1r_d[:, n0:n0 + SB, :])

            # PE warmup: ~6us of dummy matmuls under the x1/x2 DMAs flips the
            # HAM clock gate to 2.4 GHz before the corr matmuls arrive.
            with tc.tile_pool(name="pwarm", bufs=1, space="PSUM") as pwarm:
                warmps = pwarm.tile([128, 512], dt.float32, tag="warmps")
                for _ in range(14):
                    nc.tensor.matmul(warmps[:], ones128[:], warm[:],
                                     start=True, stop=True)

            Xag = persist.tile([128, 18, N], dt.bfloat16)     # [ij, bo, n] (local k=8*bo+rank)
            Xres = persist.tile([16, 18, N], dt.bfloat16)     # [ij-128, bo, n]
            X_a = persist.tile([128, NCORES, 18, N], dt.bfloat16)  # [ij, k%8, k//8, n]
            X_b = persist.tile([128, 18, N], dt.bfloat16)     # [16*(k%8)+r, k//8, n]

            # h1T pad rows must be zero; memset early (cheap, off critical path)
            h1T = [persist.tile([128, 2, N], dt.bfloat16, name=f"h1T{c}")
                   for c in range(NCH)]
            for c in range(NCH):
                nc.vector.memset(h1T[c][:], 0.0)
            h2T = persist.tile([128, 2, N], dt.bfloat16)
            nc.vector.memset(h2T[:], 0.0)
            h3T = persist.tile([128, 3, N], dt.bfloat16)
            nc.vector.memset(h3T[:], 0.0)

            # w1 stream pool opened for the whole kernel so prefetch starts
            # during the corr phase (14 bufs x 0.75 MB lookahead).
            with tc.tile_pool(name="w1p", bufs=14) as w1p:
                # ------- corr phase: k-split (this rank owns k%8==rank) -------
                with tc.tile_pool(name="csq", bufs=2) as csq, \
                     tc.tile_pool(name="pssq", bufs=1, space="PSUM") as pssq, \
                     tc.tile_pool(name="pss2", bufs=1, space="PSUM") as pss2, \
                     tc.tile_pool(name="pca", bufs=2, space="PSUM") as pca, \
                     tc.tile_pool(name="pcb", bufs=1, space="PSUM") as pcb:
                    for blk in range(NSB):
                        n0 = SB * blk
                        x1t = x1sb[:, n0:n0 + SB, :]
                        x2t = x2sb[:, n0:n0 + SB, :]

                        sq1 = csq.tile([C, SB, HW], dt.bfloat16, tag="sq1")
                        nc.vector.tensor_tensor(sq1[:], x1t, x1t, ALU.mult)
                        sq2 = csq.tile([C, SB, 18], dt.bfloat16, tag="sq2")
                        nc.vector.tensor_tensor(sq2[:], x2t, x2t, ALU.mult)

                        # rsqrt of the partition-broadcast ssq, bf16 out
                        r1 = csq.tile([128, SB, HW], dt.bfloat16, tag="r1")
                        for q in range(2):
                            b0 = 8 * q
                            ssq = pssq.tile([128, 4, 512], dt.float32, tag="ssq")
                            for h in range(4):
                                nc.tensor.matmul(ssq[:, h, 0:2 * HW], ones128[:],
                                                 sq1[:, b0 + 2 * h:b0 + 2 * h + 2, :],
                                                 start=True, stop=True)
                            nc.scalar.activation(
                                r1[:, b0:b0 + 4, :].rearrange("p (a b) k -> p a (b k)", b=2),
                                ssq[:, 0:2, 0:2 * HW], AF.Abs_reciprocal_sqrt)
                            nc.scalar.activation(
                                r1[:, b0 + 4:b0 + 8, :].rearrange("p (a b) k -> p a (b k)", b=2),
                                ssq[:, 2:4, 0:2 * HW], AF.Abs_reciprocal_sqrt)
                        ssq2 = pss2.tile([128, 512], dt.float32, tag="ssq2")
                        nc.tensor.matmul(ssq2[:, 0:SB * 18], ones128[:],
                                         sq2[:].rearrange("p n k -> p (n k)"),
                                         start=True, stop=True)
                        r2 = csq.tile([128, SB, 18], dt.bfloat16, tag="r2")
                        nc.scalar.activation(
                            r2[:].rearrange("p n k -> p (n k)"),
                            ssq2[:, 0:SB * 18], AF.Abs_reciprocal_sqrt)

                        x1s = csq.tile([C, SB, HW], dt.bfloat16, tag="x1s")
                        nc.vector.tensor_tensor(x1s[:], x1t, r1[:], ALU.mult)
                        x2s = csq.tile([C, SB, 18], dt.bfloat16, tag="x2s")
                        nc.vector.tensor_tensor(x2s[:], x2t, r2[:], ALU.mult)

                        ca = pca.tile([128, SB, 18], dt.float32, tag="ca")
                        cb = pcb.tile([16, SB, 18], dt.float32, tag="cb")
                        for b in range(SB):
                            nc.tensor.matmul(ca[:, b, :], x1s[:, b, 0:128],
                                             x2s[:, b, :], start=True, stop=True)
                            nc.tensor.matmul(cb[:, b, :], x1s[:, b, 128:HW],
                                             x2s[:, b, :], start=True, stop=True)
                        nc.vector.tensor_copy(
                            Xag[:, :, n0:n0 + SB].rearrange("p k n -> p n k"), ca[:])
                        nc.scalar.copy(
                            Xres[:, :, n0:n0 + SB].rearrange("p k n -> p n k"), cb[:])

                    # AllGather X: main part and the ij-residue. The residue
                    # gather lands exactly in the X_b layout (k%8 == source
                    # rank), so no regrouping is needed.
                    nc.scalar.dma_start(agx_in[:], Xag[:])
                    nc.scalar.dma_start(agr_in[:], Xres[:])
                    nc.gpsimd.collective_compute(
                        "AllGather", mybir.AluOpType.bypass, replica_groups=rg,
                        ins=[agx_in[:]], outs=[agx_out[:]])
                    nc.gpsimd.collective_compute(
                        "AllGather", mybir.AluOpType.bypass, replica_groups=rg,
                        ins=[agr_in[:]], outs=[agr_out[:]])
                    nc.gpsimd.dma_start(
                        X_a[:], agx_out[:].rearrange("(r p) k n -> p r k n", p=128))
                    nc.gpsimd.dma_start(X_b[:], agr_out[:])

                # ---------------- L1 (2 column chunks) + AG + L2 ----------------
                h1sb = persist.tile([64, COLS1], dt.bfloat16)
                h2sb = persist.tile([64, COLS2], dt.bfloat16)
                xt2 = [persist.tile([128, NCORES, 2, N], dt.bfloat16, name=f"xt2_{c}")
                       for c in range(NCH)]
                w2sb = [persist.tile([128, 2 * NCORES, COLS2], dt.bfloat16, name=f"w2sb{c}")
                        for c in range(NCH)]

                with tc.tile_pool(name="bias", bufs=1) as biasp, \
                     tc.tile_pool(name="ph1", bufs=1, space="PSUM") as ph1, \
                     tc.tile_pool(name="ptp", bufs=2, space="PSUM") as ptp, \
                     tc.tile_pool(name="ph2", bufs=1, space="PSUM") as ph2:
                    b1row = biasp.tile([1, COLS1], dt.bfloat16, tag="b1")
                    nc.scalar.dma_start(b1row[:], b1s_d[:, :])
                    b2row = biasp.tile([1, COLS2], dt.bfloat16, tag="b2")
                    nc.scalar.dma_start(b2row[:], b2s_d[:, :])
                    for c in range(NCH):
                        nc.scalar.dma_start(w2sb[c][:], w2c_d[c][:, :, :])
                    h1ps = [ph1.tile([64, CCOLS], dt.float32, tag=f"h1ps{c}",
                                     name=f"h1ps{c}") for c in range(NCH)]
                    h2ps = ph2.tile([64, COLS2], dt.float32, tag="h2ps")

                    for c in range(NCH):
                        for tb in range(NTB):
                            w1t = w1p.tile([128, TB, CCOLS], dt.bfloat16, tag="w1t")
                            nc.sync.dma_start(
                                w1t[:], w1c_d[c][:, TB * tb:TB * tb + TB, :])
                            for j in range(TB):
                                t = TB * tb + j
                                lhsT = (X_a[:, t % 8, t // 8, :] if t < 144
                                        else X_b[:, t - 144, :])
                                nc.tensor.matmul(h1ps[c][:], lhsT, w1t[:, j, :],
                                                 start=(t == 0), stop=False)
                        with tc.high_priority():
                            nc.tensor.matmul(h1ps[c][:], onesrow[:],
                                             b1row[:, CCOLS * c:CCOLS * (c + 1)],
                                             start=False, stop=True)
                            nc.scalar.activation(h1sb[:, CCOLS * c:CCOLS * (c + 1)],
                                                 h1ps[c][:], AF.Relu)
                            # transpose chunk -> h1T[c][p, s, n] (col = 128*s+p)
                            for t in range(2):
                                w = 128 if t < 1 else CCOLS - 128  # 88
                                tp = ptp.tile([128, 64], dt.bfloat16, tag="tp")
                                nc.tensor.transpose(
                                    tp[0:w, :],
                                    h1sb[:, CCOLS * c + 128 * t:CCOLS * c + 128 * t + w],
                                    ident[0:64, 0:64])
                                nc.vector.tensor_copy(h1T[c][0:w, t, :], tp[0:w, :])
                            nc.sync.dma_start(ag1_in[c][:], h1T[c][:])
                            nc.gpsimd.collective_compute(
                                "AllGather", mybir.AluOpType.bypass, replica_groups=rg,
                                ins=[ag1_in[c][:]], outs=[ag1_out[c][:]])

                    for c in range(NCH):
                        nc.scalar.dma_start(
                            xt2[c][:],
                            ag1_out[c][:].rearrange("(r p) s n -> p r s n", p=128))

                    # L2: accumulate both gathered chunks. tile_wait_until
                    # pushes these after the chunk tails in the modeled
                    # schedule (the scheduler underestimates AG latency and
                    # would otherwise hoist L2 before chunk 1's transposes,
                    # stalling the PE queue on the collective).
                    # zero-contribution matmul (pad rows of w2 are zero) that
                    # reads h1T[1]: forces the PE stream to finish chunk 1's
                    # transposes before starting L2, so AG-c1 launches early.
                    nc.tensor.matmul(h2ps[:], h1T[NCH - 1][:, 1, :],
                                     warm[:, 0:COLS2], start=True, stop=False)
                    for c in range(NCH):
                        for tt in range(2 * NCORES):
                            nc.tensor.matmul(h2ps[:], xt2[c][:, tt // 2, tt % 2, :],
                                             w2sb[c][:, tt, :],
                                             start=False, stop=False)
                    nc.tensor.matmul(h2ps[:], onesrow[:], b2row[:],
                                     start=False, stop=True)
                    nc.scalar.activation(h2sb[:], h2ps[:], AF.Relu)

                    # transpose h2 -> [162(+pad), 64] and AllGather
                    for t in range(2):
                        w = 128 if t < 1 else COLS2 - 128  # 34
                        tp = ptp.tile([128, 64], dt.bfloat16, tag="tp")
                        nc.tensor.transpose(tp[0:w, :], h2sb[:, 128 * t:128 * t + w],
                                            ident[0:64, 0:64])
                        nc.vector.tensor_copy(h2T[0:w, t, :], tp[0:w, :])
                    nc.sync.dma_start(ag2_in[:], h2T[:])
                    nc.gpsimd.collective_compute(
                        "AllGather", mybir.AluOpType.bypass, replica_groups=rg,
                        ins=[ag2_in[:]], outs=[ag2_out[:]])

                # ---------------- L3 (redundant) ----------------
                h3sb = persist.tile([64, 324], dt.bfloat16)
                with tc.tile_pool(name="l3", bufs=1) as l3p, \
                     tc.tile_pool(name="ph3", bufs=1, space="PSUM") as ph3:
                    w3sb = l3p.tile([128, 16, 324], dt.bfloat16, tag="w3sb")
                    nc.scalar.dma_start(w3sb[:], w3p_d[:, :, :])
                    b3row = l3p.tile([1, 324], dt.bfloat16, tag="b3")
                    nc.scalar.dma_start(b3row[:], b3_d[:, :])
                    xt3 = l3p.tile([128, NCORES, 2, N], dt.bfloat16, tag="xt3")
                    nc.gpsimd.dma_start(
                        xt3[:], ag2_out[:].rearrange("(r p) s n -> p r s n", p=128))
                    h3ps = ph3.tile([64, 324], dt.float32, tag="h3ps")
                    for t in range(16):
                        nc.tensor.matmul(h3ps[:], xt3[:, t // 2, t % 2, :], w3sb[:, t, :],
                                         start=(t == 0), stop=False)
                    nc.tensor.matmul(h3ps[:], onesrow[:], b3row[:], start=False, stop=True)
                    nc.scalar.activation(h3sb[:], h3ps[:], AF.Tanh)

                # ---------------- L4 (redundant) ----------------
                with tc.tile_pool(name="ptp3", bufs=2, space="PSUM") as ptp3, \
                     tc.tile_pool(name="l4", bufs=1) as l4p, \
                     tc.tile_pool(name="ph4", bufs=1, space="PSUM") as ph4:
                    for t in range(3):
                        w = 128 if t < 2 else 324 - 256  # 68
                        tp = ptp3.tile([128, 64], dt.bfloat16, tag="tp3")
                        nc.tensor.transpose(tp[0:w, :], h3sb[:, 128 * t:128 * t + w],
                                            ident[0:64, 0:64])
                        nc.vector.tensor_copy(h3T[0:w, t, :], tp[0:w, :])
                    w4sb = l4p.tile([128, 3, 8], dt.bfloat16, tag="w4sb")
                    nc.scalar.dma_start(w4sb[:], w4p_d[:, :, :])
                    b4row = l4p.tile([1, 8], dt.bfloat16, tag="b4")
                    nc.scalar.dma_start(b4row[:], b4_d[:, :])
                    outps = ph4.tile([64, 8], dt.float32, tag="outps")
                    for t in range(3):
                        nc.tensor.matmul(outps[:], h3T[:, t, :], w4sb[:, t, :],
                                         start=(t == 0), stop=False)
                    nc.tensor.matmul(outps[:], onesrow[:], b4row[:], start=False, stop=True)
                    outsb = l4p.tile([64, 8], dt.float32, tag="outsb")
                    nc.vector.tensor_copy(outsb[:], outps[:])
                    nc.scalar.dma_start(out_d[:, :], outsb[:])

    nc.compile()
    return nc


def _prep_inputs(x1, x2, w1, b1, w2, b2, w3, b3, w4, b4):
    """Host-side shard/permute/cast. Returns per-core input maps."""
    x1f = np.asarray(x1, np.float32).reshape(N, C, HW)
    x2f = np.asarray(x2, np.float32).reshape(N, C, HW)
    x1r = _bf16(np.ascontiguousarray(x1f.transpose(1, 0, 2)))
    x2r = _bf16(np.ascontiguousarray(x2f.transpose(1, 0, 2)))
    w1 = np.asarray(w1, np.float32)
    w2 = np.asarray(w2, np.float32)
    w3 = np.asarray(w3, np.float32)
    w4 = np.asarray(w4, np.float32)

    # w2 column-slice per core; rows padded to the chunked-AG layout:
    # chunk c, rank r, sub-tile s in 0..2, partition p ->
    #   w2 row 648*r + 324*c + 128*s + p  (zero when 128*s+p >= 324)
    w2pad = np.zeros((NCH, NCORES, 2, 128, 1296), np.float32)
    for cc in range(NCH):
        for r in range(NCORES):
            blk = w2[COLS1 * r + CCOLS * cc: COLS1 * r + CCOLS * (cc + 1)]  # [216, 1296]
            w2pad[cc, r].reshape(CPAD, 1296)[:CCOLS] = blk
    # -> per chunk: [128, 16, 1296] with tile index tt = 2*r + s
    w2t = [np.ascontiguousarray(
        w2pad[cc].reshape(2 * NCORES, 128, 1296).transpose(1, 0, 2))
        for cc in range(NCH)]

    # w3 padded to PAD2-row rank blocks, transposed to [128, 16, 324]
    w3pad = np.zeros((NCORES, PAD2, 324), np.float32)
    for r in range(NCORES):
        w3pad[r, :COLS2] = w3[COLS2 * r:COLS2 * (r + 1)]
    w3t = np.ascontiguousarray(
        w3pad.reshape(16, 128, 324).transpose(1, 0, 2))

    w4pad = np.zeros((384, 8), np.float32)
    w4pad[:324] = w4
    w4t = np.ascontiguousarray(w4pad.reshape(3, 128, 8).transpose(1, 0, 2))

    in_maps = []
    for core in range(NCORES):
        w1c = w1[:, COLS1 * core:COLS1 * (core + 1)].reshape(HW, HW, COLS1)
        main = w1c[:, 0:128, :]
        res = w1c[:, 128:HW, :].reshape(18, 8, 16, COLS1).reshape(18, 128, COLS1)
        w1full = np.concatenate([main, res], axis=0).transpose(1, 0, 2)  # [128,162,648]
        m = {
            "x1r": x1r,
            "x2r": np.ascontiguousarray(x2r[:, :, core::8]),
            "b1s": _bf16(b1[COLS1 * core:COLS1 * (core + 1)]).reshape(1, COLS1),
            "b2s": _bf16(b2[COLS2 * core:COLS2 * (core + 1)]).reshape(1, COLS2),
            "w3p": _bf16(w3t),
            "b3r": _bf16(b3).reshape(1, 324),
            "w4p": _bf16(w4t),
            "b4r": _bf16(b4).reshape(1, 8),
        }
        for cc in range(NCH):
            m[f"w1c{cc}"] = _bf16(np.ascontiguousarray(
                w1full[:, :, CCOLS * cc:CCOLS * (cc + 1)]))
            m[f"w2c{cc}"] = _bf16(np.ascontiguousarray(
                w2t[cc][:, :, COLS2 * core:COLS2 * (core + 1)]))
        in_maps.append(m)
    return in_maps


def kernel(x1, x2, w1, b1, w2, b2, w3, b3, w4, b4):
    global LAST_RESULT
    from concourse.bass_utils import run_bass_kernel_spmd

    if "nc" not in _CACHE:
        _CACHE["nc"] = _build_nc()
    nc = _CACHE["nc"]

    in_maps = _prep_inputs(x1, x2, w1, b1, w2, b2, w3, b3, w4, b4)
    trace = bool(int(os.environ.get("HNET_TRACE", "0")))
    res = run_bass_kernel_spmd(nc, in_maps, core_ids=list(range(NCORES)),
                               trace=trace)
    LAST_RESULT = res
    H = np.asarray(res.results[0]["out"], np.float32)
    ones = np.ones((N, 1), np.float32)
    return np.concatenate([H, ones], axis=1).reshape(N, 3, 3)


# revision 26
# speedup vs baseline: 1.0945x; 1.0198x over previous
"""HNetv1 Trainium2 Bass kernel (v3 — pipelined, queue-separated).

Strategy (8 NeuronCores):
  - Warmup AllGather issued at t~0 absorbs SPMD launch skew / first-collective
    rendezvous while compute proceeds.
  - PE warmup matmuls run under the x1/x2 input DMAs so the HAM clock gate
    reaches 2.4 GHz before the corr matmuls start.
  - Every core computes the l2-normalized correlation X = [64, 20736] for all
    batches (needed in full for column-parallel L1); elementwise norm work is
    batched in 16-batch superblocks to amortize per-op engine overhead.
  - w1 column-slice per core streamed from HBM in [128, 9, 216] bf16 tiles
    (per-partition-contiguous layout -> large DMA descriptors), prefetched
    during the corr phase via a deep tile pool. w1 DMAs own the Sync (SP)
    HWDGE queue; weight/bias loads go on the Scalar (ACT) HWDGE queue; loads
    that wait on collectives go on the GpSimd (SWDGE) queue so nothing blocks
    the w1 stream.
  - L1 is split into 3 column chunks of 216: each chunk's transpose+AllGather
    overlaps the next chunk's matmuls; L2 accumulates all gathered chunks
    (a zero-contribution matmul pins L2 behind the last chunk's transposes
    in the PE stream so the final AllGather launches as early as possible).
  - L2 column-split + AllGather, L3/L4 redundant on every core.

Layouts (validated against the reference in numpy):
  x1r/x2r: [C=128, N=64, HW=144] bf16 (host-transposed).
  corr^T for batch n is produced as psum [ij, k]; X_a[ij 0:128, k, n] holds the
  main part, the ij residue (16 rows) is staged in X_btmp[16, k, n] and
  regrouped by an SBUF->SBUF DMA into X_b[16*bi+r, bo, n] with k = 8*bo+bi.
  w1 per core/chunk: [128, 162, 324] where tile t<144 is w1[(k=t)*144+ij] rows
  ij 0..127 and tiles 144+bo hold the gathered residue rows — L1 is 162 plain
  [128,64]x[128,324] accumulating matmuls per chunk.
"""

import os
import numpy as np
import ml_dtypes

N, C, S = 64, 128, 12
HW = S * S            # 144
RIN = S ** 4          # 20736
NCORES = 8
COLS1 = 5184 // NCORES   # 648
COLS2 = 1296 // NCORES   # 162
G = 4                    # batches per corr psum group
NGRP = N // G            # 16
SB = 16                  # batches per corr superblock (elementwise granularity)
NSB = N // SB            # 4
NCH = 3                  # L1 column chunks
CCOLS = COLS1 // NCH     # 324
CPAD = 256               # padded h1T rows per rank per chunk (2*128)
PAD2 = 256               # per-rank padded h2T rows (2*128)
TB = 9                   # w1 k-tiles per DMA (5832B/partition descriptors)
NTB = 162 // TB          # 9 DMAs per chunk

_CACHE = {}

LAST_RESULT = None  # BassKernelResults from the most recent run (for test.py)


def _bf16(a):
    return np.asarray(a, dtype=np.float32).astype(ml_dtypes.bfloat16)


def _build_nc(trace_enabled=False):
    import concourse.bacc as bacc
    import concourse.tile as tile
    import concourse.mybir as mybir
    from concourse.masks import make_identity

    dt = mybir.dt
    AF = mybir.ActivationFunctionType
    ALU = mybir.AluOpType

    nc = bacc.Bacc("TRN2", target_bir_lowering=False, debug=False,
                   num_devices=NCORES)

    x1r_d = nc.dram_tensor("x1r", [C, N, HW], dt.bfloat16, kind="ExternalInput")
    x2r_d = nc.dram_tensor("x2r", [C, N, HW], dt.bfloat16, kind="ExternalInput")
    w1c_d = [nc.dram_tensor(f"w1c{c}", [128, 162, CCOLS], dt.bfloat16,
                            kind="ExternalInput") for c in range(NCH)]
    b1s_d = nc.dram_tensor("b1s", [1, COLS1], dt.bfloat16, kind="ExternalInput")
    w2c_d = [nc.dram_tensor(f"w2c{c}", [128, 2 * NCORES, COLS2], dt.bfloat16,
                            kind="ExternalInput") for c in range(NCH)]
    b2s_d = nc.dram_tensor("b2s", [1, COLS2], dt.bfloat16, kind="ExternalInput")
    w3p_d = nc.dram_tensor("w3p", [128, 16, 324], dt.bfloat16, kind="ExternalInput")
    b3_d = nc.dram_tensor("b3r", [1, 324], dt.bfloat16, kind="ExternalInput")
    w4p_d = nc.dram_tensor("w4p", [128, 3, 8], dt.bfloat16, kind="ExternalInput")
    b4_d = nc.dram_tensor("b4r", [1, 8], dt.bfloat16, kind="ExternalInput")
    out_d = nc.dram_tensor("out", [N, 8], dt.float32, kind="ExternalOutput")

    rg = [list(range(NCORES))]

    with tile.TileContext(nc) as tc:
        with tc.tile_pool(name="persist", bufs=1) as persist, \
             tc.tile_pool(name="dramp", bufs=1, space="DRAM") as dramp:
            # internal DRAM for collectives
            wu_in = dramp.tile([1, 16], dt.bfloat16)
            wu_out = dramp.tile([NCORES, 16], dt.bfloat16, addr_space="Shared")
            ag1_in = [dramp.tile([128, 2, N], dt.bfloat16, name=f"ag1_in{c}")
                      for c in range(NCH)]
            ag1_out = [dramp.tile([NCORES * 128, 2, N], dt.bfloat16,
                                  addr_space="Shared", name=f"ag1_out{c}")
                       for c in range(NCH)]
            ag2_in = dramp.tile([128, 2, N], dt.bfloat16)
            ag2_out = dramp.tile([NCORES * 128, 2, N], dt.bfloat16, addr_space="Shared")

            # warmup collective: absorbs SPMD launch skew + ncfw rendezvous
            # while the corr phase computes.
            wtiny = persist.tile([1, 16], dt.bfloat16)
            with tc.high_priority():
                nc.vector.memset(wtiny[:], 0.0)
                nc.gpsimd.dma_start(wu_in[:], wtiny[:])
                nc.gpsimd.collective_compute(
                    "AllGather", mybir.AluOpType.bypass, replica_groups=rg,
                    ins=[wu_in[:]], outs=[wu_out[:]])

            ones128 = persist.tile([128, 128], dt.bfloat16)
            nc.vector.memset(ones128[:], 1.0)
            onesrow = persist.tile([1, N], dt.bfloat16)
            nc.vector.memset(onesrow[:], 1.0)
            ident = persist.tile([128, 128], dt.bfloat16)
            make_identity(nc, ident[:])
            warm = persist.tile([128, 512], dt.bfloat16)
            nc.vector.memset(warm[:], 0.0)
            nc.scalar.activation(warm[0:1, 0:16], warm[0:1, 0:16], AF.Tanh)

            # full x1/x2 resident in SBUF, loaded per-superblock so corr
            # starts as soon as the first slice lands
            x1sb = persist.tile([C, N, HW], dt.bfloat16)
            x2sb = persist.tile([C, N, HW], dt.bfloat16)
            for sb in range(NSB):
                n0 = SB * sb
                nc.sync.dma_start(x1sb[:, n0:n0 + SB, :], x1r_d[:, n0:n0 + SB, :])
                nc.sync.dma_start(x2sb[:, n0:n0 + SB, :], x2r_d[:, n0:n0 + SB, :])

            # PE warmup: ~6us of dummy matmuls under the x1/x2 DMAs flips the
            # HAM clock gate to 2.4 GHz before the corr matmuls arrive.
            with tc.tile_pool(name="pwarm", bufs=1, space="PSUM") as pwarm:
                warmps = pwarm.tile([128, 512], dt.float32, tag="warmps")
                for _ in range(14):
                    nc.tensor.matmul(warmps[:], ones128[:], warm[:],
                                     start=True, stop=True)

            X_a = persist.tile([128, HW, N], dt.bfloat16)     # [ij, k, n]
            X_b = persist.tile([128, 18, N], dt.bfloat16)     # [16*bi+r, bo, n]
            X_btmp = persist.tile([16, HW, N], dt.bfloat16)   # [r, k, n]

            # h1T pad rows must be zero; memset early (cheap, off critical path)
            h1T = [persist.tile([128, 2, N], dt.bfloat16, name=f"h1T{c}")
                   for c in range(NCH)]
            for c in range(NCH):
                nc.vector.memset(h1T[c][:], 0.0)
            h2T = persist.tile([128, 2, N], dt.bfloat16)
            nc.vector.memset(h2T[:], 0.0)
            h3T = persist.tile([128, 3, N], dt.bfloat16)
            nc.vector.memset(h3T[:], 0.0)

            # w1 stream pool opened for the whole kernel so prefetch starts
            # during the corr phase (7 bufs x 0.75 MB lookahead).
            with tc.tile_pool(name="w1p", bufs=7) as w1p:
                # ---------------- corr phase ----------------
                with tc.tile_pool(name="csq", bufs=2) as csq, \
                     tc.tile_pool(name="pssq", bufs=1, space="PSUM") as pssq, \
                     tc.tile_pool(name="pca", bufs=1, space="PSUM") as pca, \
                     tc.tile_pool(name="pcb", bufs=1, space="PSUM") as pcb:
                    for blk in range(NSB):
                        n0 = SB * blk
                        x1t = x1sb[:, n0:n0 + SB, :]
                        x2t = x2sb[:, n0:n0 + SB, :]

                        sq1 = csq.tile([C, SB, HW], dt.bfloat16, tag="sq1")
                        nc.vector.tensor_tensor(sq1[:], x1t, x1t, ALU.mult)
                        sq2 = csq.tile([C, SB, HW], dt.bfloat16, tag="sq2")
                        nc.vector.tensor_tensor(sq2[:], x2t, x2t, ALU.mult)

                        # per-4-batch psum ssq; rsqrt via Abs_reciprocal_sqrt
                        # lands in bf16 superblock tiles (ssq >= 0)
                        r1 = csq.tile([128, SB, HW], dt.bfloat16, tag="r1")
                        r2 = csq.tile([128, SB, HW], dt.bfloat16, tag="r2")
                        for q in range(4):
                            b0 = 4 * q
                            ssq = pssq.tile([128, 4, 512], dt.float32, tag="ssq")
                            for h in range(2):
                                nc.tensor.matmul(ssq[:, h, 0:2 * HW], ones128[:],
                                                 sq1[:, b0 + 2 * h:b0 + 2 * h + 2, :],
                                                 start=True, stop=True)
                                nc.tensor.matmul(ssq[:, 2 + h, 0:2 * HW], ones128[:],
                                                 sq2[:, b0 + 2 * h:b0 + 2 * h + 2, :],
                                                 start=True, stop=True)
                            nc.scalar.activation(
                                r1[:, b0:b0 + 4, :].rearrange("p (a b) k -> p a (b k)", b=2),
                                ssq[:, 0:2, 0:2 * HW], AF.Abs_reciprocal_sqrt)
                            nc.scalar.activation(
                                r2[:, b0:b0 + 4, :].rearrange("p (a b) k -> p a (b k)", b=2),
                                ssq[:, 2:4, 0:2 * HW], AF.Abs_reciprocal_sqrt)

                        x1s = csq.tile([C, SB, HW], dt.bfloat16, tag="x1s")
                        nc.vector.tensor_tensor(x1s[:], x1t, r1[:], ALU.mult)
                        x2s = csq.tile([C, SB, HW], dt.bfloat16, tag="x2s")
                        nc.vector.tensor_tensor(x2s[:], x2t, r2[:], ALU.mult)

                        for q in range(4):
                            b0 = 4 * q
                            ca = pca.tile([128, 2, 512], dt.float32, tag="ca")
                            cb = pcb.tile([16, 2, 512], dt.float32, tag="cb")
                            for j in range(4):
                                b = b0 + j
                                nc.tensor.matmul(
                                    ca[:, j // 2, HW * (j % 2):HW * (j % 2) + HW],
                                    x1s[:, b, 0:128], x2s[:, b, :],
                                    start=True, stop=True)
                                nc.tensor.matmul(
                                    cb[:, j // 2, HW * (j % 2):HW * (j % 2) + HW],
                                    x1s[:, b, 128:HW], x2s[:, b, :],
                                    start=True, stop=True)
                            nb = n0 + b0
                            nc.vector.tensor_copy(
                                X_a[:, :, nb:nb + 4].rearrange("p k (a b) -> p k a b", b=2),
                                ca[:, :, 0:2 * HW].rearrange("p a (b k) -> p k a b", b=2))
                            nc.scalar.copy(
                                X_btmp[:, :, nb:nb + 4].rearrange("r k (a b) -> r k a b", b=2),
                                cb[:, :, 0:2 * HW].rearrange("r a (b k) -> r k a b", b=2))

                    # regroup residue: X_b[16*bi+r, bo, n] = X_btmp[r, 8*bo+bi, n]
                    # (scalar HWDGE queue: naturally ordered after the X_btmp copies)
                    xbt = X_btmp[:].rearrange("r (bo bi) n -> r bo bi n", bi=8)
                    for bi in range(8):
                        nc.scalar.dma_start(X_b[16 * bi:16 * bi + 16, :, :],
                                            xbt[:, :, bi, :])

                # ---------------- L1 (2 column chunks) + AG + L2 ----------------
                h1sb = persist.tile([64, COLS1], dt.bfloat16)
                h2sb = persist.tile([64, COLS2], dt.bfloat16)
                xt2 = [persist.tile([128, NCORES, 2, N], dt.bfloat16, name=f"xt2_{c}")
                       for c in range(NCH)]
                w2sb = [persist.tile([128, 2 * NCORES, COLS2], dt.bfloat16, name=f"w2sb{c}")
                        for c in range(NCH)]

                with tc.tile_pool(name="bias", bufs=1) as biasp, \
                     tc.tile_pool(name="ph1", bufs=1, space="PSUM") as ph1, \
                     tc.tile_pool(name="ptp", bufs=2, space="PSUM") as ptp, \
                     tc.tile_pool(name="ph2", bufs=1, space="PSUM") as ph2:
                    b1row = biasp.tile([1, COLS1], dt.bfloat16, tag="b1")
                    nc.scalar.dma_start(b1row[:], b1s_d[:, :])
                    b2row = biasp.tile([1, COLS2], dt.bfloat16, tag="b2")
                    nc.scalar.dma_start(b2row[:], b2s_d[:, :])
                    for c in range(NCH):
                        nc.scalar.dma_start(w2sb[c][:], w2c_d[c][:, :, :])
                    h1ps = [ph1.tile([64, CCOLS], dt.float32, tag=f"h1ps{c}",
                                     name=f"h1ps{c}") for c in range(NCH)]
                    h2ps = ph2.tile([64, COLS2], dt.float32, tag="h2ps")

                    for c in range(NCH):
                        for tb in range(NTB):
                            w1t = w1p.tile([128, TB, CCOLS], dt.bfloat16, tag="w1t")
                            nc.sync.dma_start(
                                w1t[:], w1c_d[c][:, TB * tb:TB * tb + TB, :])
                            for j in range(TB):
                                t = TB * tb + j
                                lhsT = X_a[:, t, :] if t < 144 else X_b[:, t - 144, :]
                                nc.tensor.matmul(h1ps[c][:], lhsT, w1t[:, j, :],
                                                 start=(t == 0), stop=False)
                        with tc.high_priority():
                            nc.tensor.matmul(h1ps[c][:], onesrow[:],
                                             b1row[:, CCOLS * c:CCOLS * (c + 1)],
                                             start=False, stop=True)
                            nc.scalar.activation(h1sb[:, CCOLS * c:CCOLS * (c + 1)],
                                                 h1ps[c][:], AF.Relu)
                            # transpose chunk -> h1T[c][p, s, n] (col = 128*s+p)
                            for t in range(2):
                                w = 128 if t < 1 else CCOLS - 128  # 88
                                tp = ptp.tile([128, 64], dt.bfloat16, tag="tp")
                                nc.tensor.transpose(
                                    tp[0:w, :],
                                    h1sb[:, CCOLS * c + 128 * t:CCOLS * c + 128 * t + w],
                                    ident[0:64, 0:64])
                                nc.vector.tensor_copy(h1T[c][0:w, t, :], tp[0:w, :])
                            nc.sync.dma_start(ag1_in[c][:], h1T[c][:])
                            nc.gpsimd.collective_compute(
                                "AllGather", mybir.AluOpType.bypass, replica_groups=rg,
                                ins=[ag1_in[c][:]], outs=[ag1_out[c][:]])

                    for c in range(NCH):
                        nc.scalar.dma_start(
                            xt2[c][:],
                            ag1_out[c][:].rearrange("(r p) s n -> p r s n", p=128))

                    # L2: accumulate both gathered chunks. tile_wait_until
                    # pushes these after the chunk tails in the modeled
                    # schedule (the scheduler underestimates AG latency and
                    # would otherwise hoist L2 before chunk 1's transposes,
                    # stalling the PE queue on the collective).
                    # zero-contribution matmul (pad rows of w2 are zero) that
                    # reads h1T[1]: forces the PE stream to finish chunk 1's
                    # transposes before starting L2, so AG-c1 launches early.
                    nc.tensor.matmul(h2ps[:], h1T[NCH - 1][:, 1, :],
                                     warm[:, 0:COLS2], start=True, stop=False)
                    for c in range(NCH):
                        for tt in range(2 * NCORES):
                            nc.tensor.matmul(h2ps[:], xt2[c][:, tt // 2, tt % 2, :],
                                             w2sb[c][:, tt, :],
                                             start=False, stop=False)
                    nc.tensor.matmul(h2ps[:], onesrow[:], b2row[:],
                                     start=False, stop=True)
                    nc.scalar.activation(h2sb[:], h2ps[:], AF.Relu)

                    # transpose h2 -> [162(+pad), 64] and AllGather
                    for t in range(2):
                        w = 128 if t < 1 else COLS2 - 128  # 34
                        tp = ptp.tile([128, 64], dt.bfloat16, tag="tp")
                        nc.tensor.transpose(tp[0:w, :], h2sb[:, 128 * t:128 * t + w],
                                            ident[0:64, 0:64])
                        nc.vector.tensor_copy(h2T[0:w, t, :], tp[0:w, :])
                    nc.sync.dma_start(ag2_in[:], h2T[:])
                    nc.gpsimd.collective_compute(
                        "AllGather", mybir.AluOpType.bypass, replica_groups=rg,
                        ins=[ag2_in[:]], outs=[ag2_out[:]])

                # ---------------- L3 (redundant) ----------------
                h3sb = persist.tile([64, 324], dt.bfloat16)
                with tc.tile_pool(name="l3", bufs=1) as l3p, \
                     tc.tile_pool(name="ph3", bufs=1, space="PSUM") as ph3:
                    w3sb = l3p.tile([128, 16, 324], dt.bfloat16, tag="w3sb")
                    nc.scalar.dma_start(w3sb[:], w3p_d[:, :, :])
                    b3row = l3p.tile([1, 324], dt.bfloat16, tag="b3")
                    nc.scalar.dma_start(b3row[:], b3_d[:, :])
                    xt3 = l3p.tile([128, NCORES, 2, N], dt.bfloat16, tag="xt3")
                    nc.gpsimd.dma_start(
                        xt3[:], ag2_out[:].rearrange("(r p) s n -> p r s n", p=128))
                    h3ps = ph3.tile([64, 324], dt.float32, tag="h3ps")
                    for t in range(16):
                        nc.tensor.matmul(h3ps[:], xt3[:, t // 2, t % 2, :], w3sb[:, t, :],
                                         start=(t == 0), stop=False)
                    nc.tensor.matmul(h3ps[:], onesrow[:], b3row[:], start=False, stop=True)
                    nc.scalar.activation(h3sb[:], h3ps[:], AF.Tanh)

                # ---------------- L4 (redundant) ----------------
                with tc.tile_pool(name="ptp3", bufs=2, space="PSUM") as ptp3, \
                     tc.tile_pool(name="l4", bufs=1) as l4p, \
                     tc.tile_pool(name="ph4", bufs=1, space="PSUM") as ph4:
                    for t in range(3):
                        w = 128 if t < 2 else 324 - 256  # 68
                        tp = ptp3.tile([128, 64], dt.bfloat16, tag="tp3")
                        nc.tensor.transpose(tp[0:w, :], h3sb[:, 128 * t:128 * t + w],
                                            ident[0:64, 0:64])
                        nc.vector.tensor_copy(h3T[0:w, t, :], tp[0:w, :])
                    w4sb = l4p.tile([128, 3, 8], dt.bfloat16, tag="w4sb")
                    nc.scalar.dma_start(w4sb[:], w4p_d[:, :, :])
                    b4row = l4p.tile([1, 8], dt.bfloat16, tag="b4")
                    nc.scalar.dma_start(b4row[:], b4_d[:, :])
                    outps = ph4.tile([64, 8], dt.float32, tag="outps")
                    for t in range(3):
                        nc.tensor.matmul(outps[:], h3T[:, t, :], w4sb[:, t, :],
                                         start=(t == 0), stop=False)
                    nc.tensor.matmul(outps[:], onesrow[:], b4row[:], start=False, stop=True)
                    outsb = l4p.tile([64, 8], dt.float32, tag="outsb")
                    nc.vector.tensor_copy(outsb[:], outps[:])
                    nc.scalar.dma_start(out_d[:, :], outsb[:])

    nc.compile()
    return nc


def _prep_inputs(x1, x2, w1, b1, w2, b2, w3, b3, w4, b4):
    """Host-side shard/permute/cast. Returns per-core input maps."""
    x1f = np.asarray(x1, np.float32).reshape(N, C, HW)
    x2f = np.asarray(x2, np.float32).reshape(N, C, HW)
    x1r = _bf16(np.ascontiguousarray(x1f.transpose(1, 0, 2)))
    x2r = _bf16(np.ascontiguousarray(x2f.transpose(1, 0, 2)))
    w1 = np.asarray(w1, np.float32)
    w2 = np.asarray(w2, np.float32)
    w3 = np.asarray(w3, np.float32)
    w4 = np.asarray(w4, np.float32)

    # w2 column-slice per core; rows padded to the chunked-AG layout:
    # chunk c, rank r, sub-tile s in 0..2, partition p ->
    #   w2 row 648*r + 324*c + 128*s + p  (zero when 128*s+p >= 324)
    w2pad = np.zeros((NCH, NCORES, 2, 128, 1296), np.float32)
    for cc in range(NCH):
        for r in range(NCORES):
            blk = w2[COLS1 * r + CCOLS * cc: COLS1 * r + CCOLS * (cc + 1)]  # [216, 1296]
            w2pad[cc, r].reshape(CPAD, 1296)[:CCOLS] = blk
    # -> per chunk: [128, 16, 1296] with tile index tt = 2*r + s
    w2t = [np.ascontiguousarray(
        w2pad[cc].reshape(2 * NCORES, 128, 1296).transpose(1, 0, 2))
        for cc in range(NCH)]

    # w3 padded to PAD2-row rank blocks, transposed to [128, 16, 324]
    w3pad = np.zeros((NCORES, PAD2, 324), np.float32)
    for r in range(NCORES):
        w3pad[r, :COLS2] = w3[COLS2 * r:COLS2 * (r + 1)]
    w3t = np.ascontiguousarray(
        w3pad.reshape(16, 128, 324).transpose(1, 0, 2))

    w4pad = np.zeros((384, 8), np.float32)
    w4pad[:324] = w4
    w4t = np.ascontiguousarray(w4pad.reshape(3, 128, 8).transpose(1, 0, 2))

    in_maps = []
    for core in range(NCORES):
        w1c = w1[:, COLS1 * core:COLS1 * (core + 1)].reshape(HW, HW, COLS1)
        main = w1c[:, 0:128, :]
        res = w1c[:, 128:HW, :].reshape(18, 8, 16, COLS1).reshape(18, 128, COLS1)
        w1full = np.concatenate([main, res], axis=0).transpose(1, 0, 2)  # [128,162,648]
        m = {
            "x1r": x1r, "x2r": x2r,
            "b1s": _bf16(b1[COLS1 * core:COLS1 * (core + 1)]).reshape(1, COLS1),
            "b2s": _bf16(b2[COLS2 * core:COLS2 * (core + 1)]).reshape(1, COLS2),
            "w3p": _bf16(w3t),
            "b3r": _bf16(b3).reshape(1, 324),
            "w4p": _bf16(w4t),
            "b4r": _bf16(b4).reshape(1, 8),
        }
        for cc in range(NCH):
            m[f"w1c{cc}"] = _bf16(np.ascontiguousarray(
                w1full[:, :, CCOLS * cc:CCOLS * (cc + 1)]))
            m[f"w2c{cc}"] = _bf16(np.ascontiguousarray(
                w2t[cc][:, :, COLS2 * core:COLS2 * (core + 1)]))
        in_maps.append(m)
    return in_maps


def kernel(x1, x2, w1, b1, w2, b2, w3, b3, w4, b4):
    global LAST_RESULT
    from concourse.bass_utils import run_bass_kernel_spmd

    if "nc" not in _CACHE:
        _CACHE["nc"] = _build_nc()
    nc = _CACHE["nc"]

    in_maps = _prep_inputs(x1, x2, w1, b1, w2, b2, w3, b3, w4, b4)
    trace = bool(int(os.environ.get("HNET_TRACE", "0")))
    res = run_bass_kernel_spmd(nc, in_maps, core_ids=list(range(NCORES)),
                               trace=trace)
    LAST_RESULT = res
    H = np.asarray(res.results[0]["out"], np.float32)
    ones = np.ones((N, 1), np.float32)
    return np.concatenate([H, ones], axis=1).reshape(N, 3, 3)
